# revision 34
# baseline (speedup 1.0000x reference)
"""MultiHeadAttention (8 heads, d_emb=512, d_hid=64, seq 2048, batch 8) on 8
Trainium2 NeuronCores.

Sharding: data parallel over batch — core i computes batch element i fully
(weights replicated, no collectives).

Per-core pipeline (engines overlap; ScalarE's 33.5M softmax exps are the
roofline):
  setup:   X loaded fp32 (kept for the residual), cast bf16, bounced through
           DRAM for DMA-transpose into X^T; weights cast bf16.
  Q/K:     per head-pair, heads col-stacked in the stationary operand so the
           projection matmuls run the full 128-wide array; per-partition bias
           add fused into the PSUM->SBUF eviction. Pair 0 up front; pairs 1-3
           stream through the scores PSUM slots inside the attention loop.
  V:       all heads at once (N=512), bias via a rank-1 (K=1 ones) matmul,
           stored with an appended ones column (V_aug) so the attention matmul
           also produces softmax denominators; interleaved into pair 0's loop.
  attn:    per (pair, s-half, key-tile): scores^T = K^T.T @ Q^T with both
           heads row-packed; exp(scale*x) on ScalarE straight out of PSUM into
           bf16 SBUF; ctx_aug^T += V_aug.T @ exp^T accumulated in PSUM.
           Normalization: row 64 of ctx_aug^T is the denominator; reciprocal,
           DRAM-bounce partition-broadcast, one multiply into concat^T.
  out:     out = concat^T.T @ Wo (+bo rank-1), residual add, LayerNorm via
           bn_stats/bn_aggr, DMA out.
"""

import copy
import json
import sys
import types

import numpy as np

for _p in ("/opt/trn_rl_repo", "/root/.axon_site/_ro/trn_rl_repo"):
    if _p not in sys.path:
        sys.path.append(_p)

import concourse.bass as bass
import concourse.mybir as mybir
import concourse.tile as tile

P = 128
S = 2048  # sequence length
E = 512  # embedding dim
H = 8  # heads
D = 64  # head dim
NP = H // 2  # head pairs
ST = S // P  # seq tiles
ET = E // P  # embedding tiles
SCALE = 1.0 / 8.0  # 1/sqrt(D)
LN_EPS = 1e-5
F32 = mybir.dt.float32
BF16 = mybir.dt.bfloat16
AF = mybir.ActivationFunctionType
OP = mybir.AluOpType


# --------------------------------------------------------------------------
# walrus in this build accepts only ONE sync-wait per instruction; Tile's sem
# assignment can attach several (e.g. the kernel-tail drain). Splitting the
# extra waits onto preceding NoOps on the same engine is semantically
# identical (engine streams execute in order).
def _split_waits(m, max_waits=1):
    for fn in m.get("functions", []):
        for blk in fn.get("blocks", []):
            new_insts = []
            for inst in blk.get("instructions", []):
                sync = inst.get("sync_info") or {}
                ow = sync.get("on_wait") or []
                if len(ow) > max_waits:
                    extra = ow[:-max_waits]
                    inst["sync_info"]["on_wait"] = ow[-max_waits:]
                    for ci in range(0, len(extra), max_waits):
                        nop = copy.deepcopy(inst)
                        nop["name"] = f"{inst['name']}ws{ci}"
                        nop["opcode"] = "NoOp"
                        nop["ins"] = []
                        nop["outs"] = []
                        nop["is_reset_sema"] = False
                        nop["sync_info"] = {
                            "on_update": [],
                            "on_wait": extra[ci : ci + max_waits],
                        }
                        new_insts.append(nop)
                new_insts.append(inst)
            blk["instructions"] = new_insts
    return m


def _patch_to_json(nc):
    orig = nc.to_json_bytes

    def patched(self):
        return json.dumps(_split_waits(json.loads(orig()))).encode()

    nc.to_json_bytes = types.MethodType(patched, nc)


def _bcast_ap(ap, parts):
    """[N]-shaped DRAM AP -> [parts, N] via zero-stride partition dim."""
    return bass.AP(
        tensor=ap.tensor, offset=ap.offset, ap=[[0, parts]] + list(ap.ap[-1:])
    )


def _emit_qk(nc, pool, pp, Wq_sb, Wk_sb, XT, QT, KT, bqk, psum_tag="pq"):
    for qk, wsb, qt in ((0, Wq_sb, QT), (1, Wk_sb, KT)):
        for cc in range(4):
            pq = pool.tile([P, 512], F32, tag=psum_tag, name="pq")
            for et in range(ET):
                nc.tensor.matmul(
                    pq,
                    lhsT=wsb[:, et, 2 * pp : 2 * pp + 2, :],
                    rhs=XT[:, et, cc * 512 : (cc + 1) * 512],
                    start=(et == 0),
                    stop=(et == ET - 1),
                )
            nc.vector.tensor_scalar_add(
                qt[:, pp, cc * 512 : (cc + 1) * 512], pq, bqk[:, qk, pp : pp + 1]
            )


# --------------------------------------------------------------------------
def build_nc():
    nc = bass.Bass()
    xD = nc.declare_dram_parameter("x", [S, E], F32, isOutput=False)
    bvD = nc.declare_dram_parameter("bv", [H, D], F32, isOutput=False)
    boD = nc.declare_dram_parameter("bo", [E], F32, isOutput=False)
    gammaD = nc.declare_dram_parameter("gamma", [E], F32, isOutput=False)
    betaD = nc.declare_dram_parameter("beta", [E], F32, isOutput=False)
    # host-preprocessed layouts: x^T and e-major weights, already bf16
    xTD = nc.declare_dram_parameter("xT", [E, S], BF16, isOutput=False)
    wqpD = nc.declare_dram_parameter("Wq_p", [E, H * D], BF16, isOutput=False)
    wkpD = nc.declare_dram_parameter("Wk_p", [E, H * D], BF16, isOutput=False)
    wvpD = nc.declare_dram_parameter("Wv_p", [E, H * D], BF16, isOutput=False)
    wopD = nc.declare_dram_parameter("Wo_p", [H * D, E], BF16, isOutput=False)
    bqkD = nc.declare_dram_parameter("bqk", [P, 2, NP], F32, isOutput=False)
    outD = nc.declare_dram_parameter("out", [S, E], F32, isOutput=True)

    with tile.TileContext(nc) as tc:
        with (
            tc.tile_pool(name="persist", bufs=1) as persist,
            tc.tile_pool(name="dramp", bufs=2, space="DRAM") as dramp,
        ):
            X = persist.tile([P, ST, E], F32, name="Xsb")
            XT = persist.tile([P, ET, S], BF16, name="XTsb")
            Wq_sb = persist.tile([P, ET, H, D], BF16, name="Wq_sb")
            Wk_sb = persist.tile([P, ET, H, D], BF16, name="Wk_sb")
            Wv_sb = persist.tile([P, ET, H, D], BF16, name="Wv_sb")
            Wo_sb = persist.tile([P, ET, E], BF16, name="Wo_sb")
            bqk = persist.tile([P, 2, NP], F32, name="bqk")
            bv_bc = persist.tile([P, H, D], F32, name="bv_bc")
            bo_row = persist.tile([1, E], BF16, name="bo_row")
            bo_stg = persist.tile([1, E], F32, name="bo_stg")
            ones_bf = persist.tile([1, P], BF16, name="ones_bf")
            gamma_bc = persist.tile([P, E], F32, name="gamma_bc")
            beta_bc = persist.tile([P, E], F32, name="beta_bc")
            QT = persist.tile([P, NP, S], BF16, name="QTsb")
            KT = persist.tile([P, NP, S], BF16, name="KTsb")
            Vaug = persist.tile([P, ST, H, D + 1], BF16, name="Vaug")
            CCT = persist.tile([P, NP, S], BF16, name="CCTsb")

            # ---------------- stage 0: direct loads (host pre-layouts) -------
            with (
                tc.tile_pool(name="qkp", bufs=2, space="PSUM") as qkp,
            ):
                nc.vector.memset(Vaug[:, :, :, D : D + 1], 1.0)
                nc.vector.memset(ones_bf, 1.0)

                # critical chain first: x^T, Wq/Wk, biases -> pair-0 Q/K
                for et in range(ET):
                    nc.sync.dma_start(
                        out=XT[:, et], in_=xTD[et * P : (et + 1) * P, :]
                    )
                for wD, wsb in ((wqpD, Wq_sb), (wkpD, Wk_sb)):
                    nc.sync.dma_start(
                        out=wsb,
                        in_=wD[:].rearrange("(et p) hd -> p et hd", p=P).rearrange(
                            "p et (h d) -> p et h d", h=H
                        ),
                    )
                nc.sync.dma_start(out=bqk, in_=bqkD[:])
                _emit_qk(nc, qkp, 0, Wq_sb, Wk_sb, XT, QT, KT, bqk)

                # the rest, off the critical queue
                nc.sync.dma_start(
                    out=Wv_sb,
                    in_=wvpD[:].rearrange("(et p) hd -> p et hd", p=P).rearrange(
                        "p et (h d) -> p et h d", h=H
                    ),
                )
                xDr = xD[:].rearrange("(st p) e -> p st e", p=P)
                for q in range(4):
                    nc.gpsimd.dma_start(
                        out=X[:, 4 * q : 4 * q + 4], in_=xDr[:, 4 * q : 4 * q + 4]
                    )
                nc.gpsimd.dma_start(
                    out=bv_bc.rearrange("p h d -> p (h d)"),
                    in_=_bcast_ap(bvD[:].rearrange("h d -> (h d)"), P),
                )
                nc.gpsimd.dma_start(out=bo_stg, in_=boD[:][None, :])
                nc.gpsimd.tensor_copy(out=bo_row, in_=bo_stg)
                tc.cur_priority += 4000
                nc.gpsimd.dma_start(
                    out=Wo_sb, in_=wopD[:].rearrange("(kt p) e -> p kt e", p=P)
                )
                for dram, sb in ((gammaD, gamma_bc), (betaD, beta_bc)):
                    nc.gpsimd.dma_start(out=sb, in_=_bcast_ap(dram[:], P))
                tc.cur_priority -= 4000

            # ---------------- stage 2: attention ----------------
            with (
                tc.tile_pool(name="expp", bufs=6) as expp,
                tc.tile_pool(name="scp", bufs=2, space="PSUM") as scp,
                tc.tile_pool(name="ctxp", bufs=2, space="PSUM") as ctxp,
                tc.tile_pool(name="smallp", bufs=2) as smallp,
                tc.tile_pool(name="outp", bufs=3) as outp,
                tc.tile_pool(name="statp", bufs=4) as statp,
            ):
                # deferred work, interleaved through the scores PSUM slots:
                # V tiles during (pair0, sh0); Q/K of pair p+1 during later blocks
                def v_chunk(st):
                    def emit():
                        pv = scp.tile([P, 1024], F32, tag="SC", name="pv")
                        for et in range(ET):
                            nc.tensor.matmul(
                                pv[:, 0:512],
                                lhsT=XT[:, et, st * P : (st + 1) * P],
                                rhs=Wv_sb[:, et],
                                start=(et == 0),
                                stop=(et == ET - 1),
                            )
                        nc.vector.tensor_tensor(
                            Vaug[:, st, :, 0:D],
                            pv[:, 0:512].rearrange("p (h d) -> p h d", h=H),
                            bv_bc,
                            OP.add,
                        )

                    return emit

                def qk_chunk(pp, qk, cc):
                    def emit():
                        wsb = Wq_sb if qk == 0 else Wk_sb
                        qt = QT if qk == 0 else KT
                        pq = scp.tile([P, 1024], F32, tag="SC", name="pq2")
                        for et in range(ET):
                            nc.tensor.matmul(
                                pq[:, 0:512],
                                lhsT=wsb[:, et, 2 * pp : 2 * pp + 2, :],
                                rhs=XT[:, et, cc * 512 : (cc + 1) * 512],
                                start=(et == 0),
                                stop=(et == ET - 1),
                            )
                        nc.vector.tensor_scalar_add(
                            qt[:, pp, cc * 512 : (cc + 1) * 512],
                            pq[:, 0:512],
                            bqk[:, qk, pp : pp + 1],
                        )

                    return emit

                # explicit chunk schedule (sh-outer block order):
                # every chunk lands before its first consumer, spread so PE
                # stays under ScalarE's per-iteration budget
                Q, K = 0, 1

                def st3_chunk(st):
                    def emit():
                        po = scp.tile([P, 1024], F32, tag="SC", name="po3")
                        for kt in range(ET):
                            nc.tensor.matmul(
                                po[:, 0:E],
                                lhsT=CCT[:, kt, st * P : (st + 1) * P],
                                rhs=Wo_sb[:, kt],
                                start=(kt == 0),
                                stop=False,
                            )
                        nc.tensor.matmul(
                            po[:, 0:E], lhsT=ones_bf, rhs=bo_row, start=False, stop=True
                        )
                        y = outp.tile([P, E], F32, tag="y", name="y")
                        nc.vector.tensor_add(y, po[:, 0:E], X[:, st])
                        stats = statp.tile([P, 6], F32, tag="stats", name="stats")
                        nc.vector.bn_stats(out=stats, in_=y)
                        mv = statp.tile([P, 2], F32, tag="mv", name="mv")
                        nc.vector.bn_aggr(out=mv, in_=stats)
                        rstd = statp.tile([P, 1], F32, tag="rstd", name="rstd")
                        nc.scalar.activation(
                            out=rstd, in_=mv[:, 1:2], func=AF.Ln, bias=eps_t
                        )
                        nc.scalar.activation(
                            out=rstd, in_=rstd, func=AF.Exp, scale=-0.5
                        )
                        nc.vector.tensor_scalar(
                            y, y, mv[:, 0:1], rstd, OP.subtract, OP.mult
                        )
                        nc.vector.tensor_tensor(y, y, gamma_bc, OP.mult)
                        nc.vector.tensor_tensor(y, y, beta_bc, OP.add)
                        nc.sync.dma_start(out=outD[st * P : (st + 1) * P, :], in_=y)

                    return emit

                sched = {
                    0: [(t, v_chunk(t)) for t in range(1, 16)]
                    + [
                        (13, qk_chunk(1, Q, 0)),
                        (14, qk_chunk(1, Q, 1)),
                        (15, qk_chunk(1, K, 0)),
                    ],
                    1: [
                        (2, qk_chunk(1, K, 1)),
                        (4, qk_chunk(1, K, 2)),
                        (6, qk_chunk(1, K, 3)),
                        (8, qk_chunk(2, Q, 0)),
                        (10, qk_chunk(2, Q, 1)),
                        (12, qk_chunk(2, K, 0)),
                        (14, qk_chunk(2, K, 1)),
                    ],
                    2: [
                        (2, qk_chunk(2, K, 2)),
                        (4, qk_chunk(2, K, 3)),
                        (6, qk_chunk(3, Q, 0)),
                        (8, qk_chunk(3, Q, 1)),
                        (10, qk_chunk(3, K, 0)),
                        (12, qk_chunk(3, K, 1)),
                        (14, qk_chunk(3, K, 2)),
                    ],
                    3: [
                        (2, qk_chunk(3, K, 3)),
                        (4, qk_chunk(1, Q, 2)),
                        (6, qk_chunk(1, Q, 3)),
                        (8, qk_chunk(2, Q, 2)),
                        (10, qk_chunk(2, Q, 3)),
                        (12, qk_chunk(3, Q, 2)),
                        (14, qk_chunk(3, Q, 3)),
                    ],
                    4: [(3, st3_chunk(0)), (7, st3_chunk(1)), (11, st3_chunk(2))],
                    5: [(3, st3_chunk(3)), (7, st3_chunk(4)), (11, st3_chunk(5))],
                    6: [(3, st3_chunk(6)), (9, st3_chunk(7))],
                    7: [],
                }

                eps_t = statp.tile([P, 1], F32, tag="eps", bufs=1)
                nc.vector.memset(eps_t, LN_EPS)

                for sh in range(2):
                    for pp in range(NP):
                        s0 = sh * 1024
                        blk = sh * NP + pp
                        slots = {}
                        for t, fn in sched.get(blk, []):
                            slots.setdefault(t, []).append(fn)
                        if blk == 0:
                            # V tile 0 before the loop: t=0's ctx needs it
                            v_chunk(0)()
                        cx = None
                        for t in range(ST):
                            for fn in slots.get(t, []):
                                fn()
                            sc = [
                                scp.tile([P, 1024], F32, tag="SC", name=f"sc{_hl}")
                                for _hl in range(2)
                            ]
                            for hl in range(2):
                                lo, hi = D * hl, D * (hl + 1)
                                for cc in range(2):
                                    nc.tensor.matmul(
                                        sc[hl][:, cc * 512 : (cc + 1) * 512],
                                        lhsT=KT[lo:hi, pp, t * P : (t + 1) * P],
                                        rhs=QT[
                                            lo:hi,
                                            pp,
                                            s0 + cc * 512 : s0 + (cc + 1) * 512,
                                        ],
                                        start=True,
                                        stop=True,
                                    )
                            et_t = expp.tile([P, 2048], BF16, tag="expT", name="et_t")
                            for hl in range(2):
                                nc.scalar.activation(
                                    out=et_t[:, hl * 1024 : (hl + 1) * 1024],
                                    in_=sc[hl],
                                    func=AF.Exp,
                                    scale=SCALE,
                                )
                            if cx is None:
                                # allocated after t=0's scores/exp so this
                                # block's first scores don't wait on the
                                # previous block's normalize chain
                                cx = [
                                    ctxp.tile(
                                        [D + 1, 1024], F32, tag="ctx", name=f"cx{_hl}"
                                    )
                                    for _hl in range(2)
                                ]
                            for hl in range(2):
                                h = 2 * pp + hl
                                for cc in range(2):
                                    nc.tensor.matmul(
                                        cx[hl][:, cc * 512 : (cc + 1) * 512],
                                        lhsT=Vaug[:, t, h, :],
                                        rhs=et_t[
                                            :,
                                            hl * 1024
                                            + cc * 512 : hl * 1024
                                            + (cc + 1) * 512,
                                        ],
                                        start=(t == 0),
                                        stop=(t == ST - 1),
                                    )
                        # softmax normalization: row D of cx is the denominator.
                        # reciprocal -> DRAM bounce -> zero-stride broadcast back
                        for hl in range(2):
                            rec = smallp.tile([P, 1024], F32, tag="rec", name="rec")
                            nc.vector.reciprocal(
                                rec[D : D + 1, :], cx[hl][D : D + 1, :]
                            )
                            dden = dramp.tile([1, 1024], F32, tag="dden", name="dden")
                            nc.sync.dma_start(out=dden, in_=rec[D : D + 1, :])
                            dbc = smallp.tile([D, 1024], F32, tag="dbc", name="dbc")
                            nc.gpsimd.dma_start(out=dbc, in_=_bcast_ap(dden[0], D))
                            if hl == 0:
                                nc.vector.tensor_tensor(
                                    CCT[0:D, pp, s0 : s0 + 1024],
                                    cx[hl][0:D, :],
                                    dbc,
                                    OP.mult,
                                )
                            else:
                                # result must land on partitions 64..127; DVE
                                # cannot shift partitions, DMA can.
                                tmp = smallp.tile(
                                    [D, 1024], BF16, tag="tmp", name="tmp"
                                )
                                nc.vector.tensor_tensor(
                                    tmp, cx[hl][0:D, :], dbc, OP.mult
                                )
                                nc.sync.dma_start(
                                    out=CCT[D : 2 * D, pp, s0 : s0 + 1024], in_=tmp
                                )

            # ---------------- stage 3: Wo, residual, LayerNorm ----------------
            with (
                tc.tile_pool(name="outp3", bufs=6) as outp3,
                tc.tile_pool(name="ps3", bufs=6, space="PSUM") as ps3,
                tc.tile_pool(name="statp3", bufs=8) as statp3,
            ):
                eps_t = statp3.tile([P, 1], F32, tag="eps", bufs=1, name="eps_t3")
                nc.vector.memset(eps_t, LN_EPS)
                # deprioritized: fills engine-idle slots during the last
                # attention block instead of starving its scores
                tc.cur_priority += 20000
                for st in range(8, ST):
                    po = ps3.tile([P, E], F32, tag="po", name="po")
                    for kt in range(ET):
                        nc.tensor.matmul(
                            po,
                            lhsT=CCT[:, kt, st * P : (st + 1) * P],
                            rhs=Wo_sb[:, kt],
                            start=(kt == 0),
                            stop=False,
                        )
                    nc.tensor.matmul(
                        po, lhsT=ones_bf, rhs=bo_row, start=False, stop=True
                    )
                    y = outp3.tile([P, E], F32, tag="y", name="y")
                    nc.vector.tensor_add(y, po, X[:, st])
                    stats = statp3.tile([P, 6], F32, tag="stats", name="stats")
                    nc.vector.bn_stats(out=stats, in_=y)
                    mv = statp3.tile([P, 2], F32, tag="mv", name="mv")
                    nc.vector.bn_aggr(out=mv, in_=stats)
                    rstd = statp3.tile([P, 1], F32, tag="rstd", name="rstd")
                    # rstd = exp(-0.5*ln(var+eps)): Ln and Exp share one ACT
                    # table set, so no table reloads between softmax exps
                    nc.scalar.activation(
                        out=rstd, in_=mv[:, 1:2], func=AF.Ln, bias=eps_t
                    )
                    nc.scalar.activation(out=rstd, in_=rstd, func=AF.Exp, scale=-0.5)
                    nc.vector.tensor_scalar(
                        y, y, mv[:, 0:1], rstd, OP.subtract, OP.mult
                    )
                    nc.vector.tensor_tensor(y, y, gamma_bc, OP.mult)
                    nc.vector.tensor_tensor(y, y, beta_bc, OP.add)
                    nc.sync.dma_start(out=outD[st * P : (st + 1) * P, :], in_=y)
                tc.cur_priority -= 20000

    _patch_to_json(nc)
    return nc


_NC_CACHE = None


def _get_nc():
    global _NC_CACHE
    if _NC_CACHE is None:
        _NC_CACHE = build_nc()
    return _NC_CACHE


def kernel(**inputs) -> np.ndarray:
    import ml_dtypes
    from concourse.bass_utils import run_bass_kernel_spmd

    BF = ml_dtypes.bfloat16
    nc = _get_nc()
    x = np.asarray(inputs["x"], dtype=np.float32)
    B = x.shape[0]

    def f32(k):
        return np.ascontiguousarray(np.asarray(inputs[k], dtype=np.float32))

    def perm_w(k):  # [H, E, D] -> [E, H*D] bf16
        w = np.asarray(inputs[k], dtype=np.float32)
        return np.ascontiguousarray(w.transpose(1, 0, 2).reshape(E, H * D).astype(BF))

    bqk = np.ascontiguousarray(
        np.stack(
            [
                np.asarray(inputs["bq"], np.float32).reshape(NP, P).T,
                np.asarray(inputs["bk"], np.float32).reshape(NP, P).T,
            ],
            axis=1,
        )
    )
    shared = {
        "Wq_p": perm_w("Wq"),
        "Wk_p": perm_w("Wk"),
        "Wv_p": perm_w("Wv"),
        "Wo_p": np.ascontiguousarray(
            np.asarray(inputs["Wo"], np.float32).astype(BF)
        ),
        "bqk": bqk,
        "bv": f32("bv"),
        "bo": f32("bo"),
        "gamma": f32("gamma"),
        "beta": f32("beta"),
    }
    in_maps = []
    for b in range(B):
        xb = np.ascontiguousarray(x[b])
        in_maps.append(
            {
                "x": xb,
                "xT": np.ascontiguousarray(xb.T.astype(BF)),
                **shared,
            }
        )
    res = run_bass_kernel_spmd(nc, in_maps, core_ids=list(range(B)))
    return np.stack([res.results[b]["out"] for b in range(B)], axis=0)


# revision 37
# speedup vs baseline: 1.0183x; 1.0183x over previous
"""MultiHeadAttention (8 heads, d_emb=512, d_hid=64, seq 2048, batch 8) on 8
Trainium2 NeuronCores.

Sharding: data parallel over batch — core i computes batch element i fully
(weights replicated, no collectives).

Per-core pipeline (engines overlap; ScalarE's 33.5M softmax exps are the
roofline):
  setup:   X loaded fp32 (kept for the residual), cast bf16, bounced through
           DRAM for DMA-transpose into X^T; weights cast bf16.
  Q/K:     per head-pair, heads col-stacked in the stationary operand so the
           projection matmuls run the full 128-wide array; per-partition bias
           add fused into the PSUM->SBUF eviction. Pair 0 up front; pairs 1-3
           stream through the scores PSUM slots inside the attention loop.
  V:       all heads at once (N=512), bias via a rank-1 (K=1 ones) matmul,
           stored with an appended ones column (V_aug) so the attention matmul
           also produces softmax denominators; interleaved into pair 0's loop.
  attn:    per (pair, s-half, key-tile): scores^T = K^T.T @ Q^T with both
           heads row-packed; exp(scale*x) on ScalarE straight out of PSUM into
           bf16 SBUF; ctx_aug^T += V_aug.T @ exp^T accumulated in PSUM.
           Normalization: row 64 of ctx_aug^T is the denominator; reciprocal,
           DRAM-bounce partition-broadcast, one multiply into concat^T.
  out:     out = concat^T.T @ Wo (+bo rank-1), residual add, LayerNorm via
           bn_stats/bn_aggr, DMA out.
"""

import copy
import json
import sys
import types

import numpy as np

for _p in ("/opt/trn_rl_repo", "/root/.axon_site/_ro/trn_rl_repo"):
    if _p not in sys.path:
        sys.path.append(_p)

import concourse.bass as bass
import concourse.mybir as mybir
import concourse.tile as tile

P = 128
S = 2048  # sequence length
E = 512  # embedding dim
H = 8  # heads
D = 64  # head dim
NP = H // 2  # head pairs
ST = S // P  # seq tiles
ET = E // P  # embedding tiles
SCALE = 1.0 / 8.0  # 1/sqrt(D)
LN_EPS = 1e-5
F32 = mybir.dt.float32
BF16 = mybir.dt.bfloat16
AF = mybir.ActivationFunctionType
OP = mybir.AluOpType


# --------------------------------------------------------------------------
# walrus in this build accepts only ONE sync-wait per instruction; Tile's sem
# assignment can attach several (e.g. the kernel-tail drain). Splitting the
# extra waits onto preceding NoOps on the same engine is semantically
# identical (engine streams execute in order).
def _split_waits(m, max_waits=1):
    for fn in m.get("functions", []):
        for blk in fn.get("blocks", []):
            new_insts = []
            for inst in blk.get("instructions", []):
                sync = inst.get("sync_info") or {}
                ow = sync.get("on_wait") or []
                if len(ow) > max_waits:
                    extra = ow[:-max_waits]
                    inst["sync_info"]["on_wait"] = ow[-max_waits:]
                    for ci in range(0, len(extra), max_waits):
                        nop = copy.deepcopy(inst)
                        nop["name"] = f"{inst['name']}ws{ci}"
                        nop["opcode"] = "NoOp"
                        nop["ins"] = []
                        nop["outs"] = []
                        nop["is_reset_sema"] = False
                        nop["sync_info"] = {
                            "on_update": [],
                            "on_wait": extra[ci : ci + max_waits],
                        }
                        new_insts.append(nop)
                new_insts.append(inst)
            blk["instructions"] = new_insts
    return m


def _patch_to_json(nc):
    orig = nc.to_json_bytes

    def patched(self):
        return json.dumps(_split_waits(json.loads(orig()))).encode()

    nc.to_json_bytes = types.MethodType(patched, nc)


def _bcast_ap(ap, parts):
    """[N]-shaped DRAM AP -> [parts, N] via zero-stride partition dim."""
    return bass.AP(
        tensor=ap.tensor, offset=ap.offset, ap=[[0, parts]] + list(ap.ap[-1:])
    )


def _emit_qk(nc, pool, pp, Wq_sb, Wk_sb, XT, QT, KT, bqk, psum_tag="pq"):
    for qk, wsb, qt in ((0, Wq_sb, QT), (1, Wk_sb, KT)):
        for cc in range(4):
            pq = pool.tile([P, 512], F32, tag=psum_tag, name="pq")
            for et in range(ET):
                nc.tensor.matmul(
                    pq,
                    lhsT=wsb[:, et, 2 * pp : 2 * pp + 2, :],
                    rhs=XT[:, et, cc * 512 : (cc + 1) * 512],
                    start=(et == 0),
                    stop=(et == ET - 1),
                )
            nc.vector.tensor_scalar_add(
                qt[:, pp, cc * 512 : (cc + 1) * 512], pq, bqk[:, qk, pp : pp + 1]
            )


# --------------------------------------------------------------------------
def build_nc():
    nc = bass.Bass()
    xD = nc.declare_dram_parameter("x", [S, E], F32, isOutput=False)
    bvD = nc.declare_dram_parameter("bv", [H, D], F32, isOutput=False)
    boD = nc.declare_dram_parameter("bo", [E], F32, isOutput=False)
    gammaD = nc.declare_dram_parameter("gamma", [E], F32, isOutput=False)
    betaD = nc.declare_dram_parameter("beta", [E], F32, isOutput=False)
    # host-preprocessed layouts: x^T and e-major weights, already bf16
    xTD = nc.declare_dram_parameter("xT", [E, S], BF16, isOutput=False)
    wqpD = nc.declare_dram_parameter("Wq_p", [E, H * D], BF16, isOutput=False)
    wkpD = nc.declare_dram_parameter("Wk_p", [E, H * D], BF16, isOutput=False)
    wvpD = nc.declare_dram_parameter("Wv_p", [E, H * D], BF16, isOutput=False)
    wopD = nc.declare_dram_parameter("Wo_p", [H * D, E], BF16, isOutput=False)
    bqkD = nc.declare_dram_parameter("bqk", [P, 2, NP], F32, isOutput=False)
    outD = nc.declare_dram_parameter("out", [S, E], F32, isOutput=True)

    with tile.TileContext(nc) as tc:
        with (
            tc.tile_pool(name="persist", bufs=1) as persist,
            tc.tile_pool(name="dramp", bufs=2, space="DRAM") as dramp,
        ):
            X = persist.tile([P, ST, E], F32, name="Xsb")
            XT = persist.tile([P, ET, S], BF16, name="XTsb")
            Wq_sb = persist.tile([P, ET, H, D], BF16, name="Wq_sb")
            Wk_sb = persist.tile([P, ET, H, D], BF16, name="Wk_sb")
            Wv_sb = persist.tile([P, ET, H, D], BF16, name="Wv_sb")
            Wo_sb = persist.tile([P, ET, E], BF16, name="Wo_sb")
            bqk = persist.tile([P, 2, NP], F32, name="bqk")
            bv_bc = persist.tile([P, H, D], F32, name="bv_bc")
            bo_row = persist.tile([1, E], BF16, name="bo_row")
            bo_stg = persist.tile([1, E], F32, name="bo_stg")
            ones_bf = persist.tile([1, P], BF16, name="ones_bf")
            gamma_bc = persist.tile([P, E], F32, name="gamma_bc")
            beta_bc = persist.tile([P, E], F32, name="beta_bc")
            QT = persist.tile([P, NP, S], BF16, name="QTsb")
            KT = persist.tile([P, NP, S], BF16, name="KTsb")
            Vaug = persist.tile([P, ST, H, D + 1], BF16, name="Vaug")
            CCT = persist.tile([P, NP, S], BF16, name="CCTsb")

            # ---------------- stage 0: direct loads (host pre-layouts) -------
            with (
                tc.tile_pool(name="qkp", bufs=2, space="PSUM") as qkp,
            ):
                nc.vector.memset(Vaug[:, :, :, D : D + 1], 1.0)
                nc.vector.memset(ones_bf, 1.0)

                # PE warmup during the initial DMA wait: HAM un-throttles
                # after ~3.4us of sustained activity, so the first real
                # matmuls (pair-0 Q/K) run at full clock instead of 1/2
                warm = qkp.tile([P, 512], F32, tag="pq", name="warm")
                for _w in range(180):
                    nc.tensor.matmul(
                        warm[:, 0:64], lhsT=ones_bf, rhs=ones_bf[:, 0:64],
                        start=True, stop=True,
                    )

                # critical chain first: x^T, Wq/Wk, biases -> pair-0 Q/K
                for et in range(ET):
                    nc.sync.dma_start(
                        out=XT[:, et], in_=xTD[et * P : (et + 1) * P, :]
                    )
                for wD, wsb in ((wqpD, Wq_sb), (wkpD, Wk_sb)):
                    nc.sync.dma_start(
                        out=wsb,
                        in_=wD[:].rearrange("(et p) hd -> p et hd", p=P).rearrange(
                            "p et (h d) -> p et h d", h=H
                        ),
                    )
                nc.sync.dma_start(out=bqk, in_=bqkD[:])
                _emit_qk(nc, qkp, 0, Wq_sb, Wk_sb, XT, QT, KT, bqk)

                # the rest, off the critical queue
                nc.sync.dma_start(
                    out=Wv_sb,
                    in_=wvpD[:].rearrange("(et p) hd -> p et hd", p=P).rearrange(
                        "p et (h d) -> p et h d", h=H
                    ),
                )

                nc.gpsimd.dma_start(
                    out=bv_bc.rearrange("p h d -> p (h d)"),
                    in_=_bcast_ap(bvD[:].rearrange("h d -> (h d)"), P),
                )
                nc.gpsimd.dma_start(out=bo_stg, in_=boD[:][None, :])
                nc.gpsimd.tensor_copy(out=bo_row, in_=bo_stg)

            # ---------------- stage 2: attention ----------------
            with (
                tc.tile_pool(name="expp", bufs=6) as expp,
                tc.tile_pool(name="scp", bufs=2, space="PSUM") as scp,
                tc.tile_pool(name="ctxp", bufs=2, space="PSUM") as ctxp,
                tc.tile_pool(name="smallp", bufs=2) as smallp,
                tc.tile_pool(name="outp", bufs=3) as outp,
                tc.tile_pool(name="statp", bufs=4) as statp,
            ):
                # deferred work, interleaved through the scores PSUM slots:
                # V tiles during (pair0, sh0); Q/K of pair p+1 during later blocks
                def v_chunk(st):
                    def emit():
                        pv = scp.tile([P, 1024], F32, tag="SC", name="pv")
                        for et in range(ET):
                            nc.tensor.matmul(
                                pv[:, 0:512],
                                lhsT=XT[:, et, st * P : (st + 1) * P],
                                rhs=Wv_sb[:, et],
                                start=(et == 0),
                                stop=(et == ET - 1),
                            )
                        nc.vector.tensor_tensor(
                            Vaug[:, st, :, 0:D],
                            pv[:, 0:512].rearrange("p (h d) -> p h d", h=H),
                            bv_bc,
                            OP.add,
                        )

                    return emit

                def qk_chunk(pp, qk, cc):
                    def emit():
                        wsb = Wq_sb if qk == 0 else Wk_sb
                        qt = QT if qk == 0 else KT
                        pq = scp.tile([P, 1024], F32, tag="SC", name="pq2")
                        for et in range(ET):
                            nc.tensor.matmul(
                                pq[:, 0:512],
                                lhsT=wsb[:, et, 2 * pp : 2 * pp + 2, :],
                                rhs=XT[:, et, cc * 512 : (cc + 1) * 512],
                                start=(et == 0),
                                stop=(et == ET - 1),
                            )
                        nc.vector.tensor_scalar_add(
                            qt[:, pp, cc * 512 : (cc + 1) * 512],
                            pq[:, 0:512],
                            bqk[:, qk, pp : pp + 1],
                        )

                    return emit

                # explicit chunk schedule (sh-outer block order):
                # every chunk lands before its first consumer, spread so PE
                # stays under ScalarE's per-iteration budget
                Q, K = 0, 1

                def st3_chunk(st):
                    def emit():
                        po = scp.tile([P, 1024], F32, tag="SC", name="po3")
                        for kt in range(ET):
                            nc.tensor.matmul(
                                po[:, 0:E],
                                lhsT=CCT[:, kt, st * P : (st + 1) * P],
                                rhs=Wo_sb[:, kt],
                                start=(kt == 0),
                                stop=False,
                            )
                        nc.tensor.matmul(
                            po[:, 0:E], lhsT=ones_bf, rhs=bo_row, start=False, stop=True
                        )
                        y = outp.tile([P, E], F32, tag="y", name="y")
                        nc.vector.tensor_add(y, po[:, 0:E], X[:, st])
                        stats = statp.tile([P, 6], F32, tag="stats", name="stats")
                        nc.vector.bn_stats(out=stats, in_=y)
                        mv = statp.tile([P, 2], F32, tag="mv", name="mv")
                        nc.vector.bn_aggr(out=mv, in_=stats)
                        rstd = statp.tile([P, 1], F32, tag="rstd", name="rstd")
                        nc.scalar.activation(
                            out=rstd, in_=mv[:, 1:2], func=AF.Ln, bias=eps_t
                        )
                        nc.scalar.activation(
                            out=rstd, in_=rstd, func=AF.Exp, scale=-0.5
                        )
                        nc.vector.tensor_scalar(
                            y, y, mv[:, 0:1], rstd, OP.subtract, OP.mult
                        )
                        nc.vector.tensor_tensor(y, y, gamma_bc, OP.mult)
                        nc.vector.tensor_tensor(y, y, beta_bc, OP.add)
                        nc.sync.dma_start(out=outD[st * P : (st + 1) * P, :], in_=y)

                    return emit

                sched = {
                    0: [(t, v_chunk(t)) for t in range(1, 16)]
                    + [
                        (13, qk_chunk(1, Q, 0)),
                        (14, qk_chunk(1, Q, 1)),
                        (15, qk_chunk(1, K, 0)),
                    ],
                    1: [
                        (2, qk_chunk(1, K, 1)),
                        (4, qk_chunk(1, K, 2)),
                        (6, qk_chunk(1, K, 3)),
                        (8, qk_chunk(2, Q, 0)),
                        (10, qk_chunk(2, Q, 1)),
                        (12, qk_chunk(2, K, 0)),
                        (14, qk_chunk(2, K, 1)),
                    ],
                    2: [
                        (2, qk_chunk(2, K, 2)),
                        (4, qk_chunk(2, K, 3)),
                        (6, qk_chunk(3, Q, 0)),
                        (8, qk_chunk(3, Q, 1)),
                        (10, qk_chunk(3, K, 0)),
                        (12, qk_chunk(3, K, 1)),
                        (14, qk_chunk(3, K, 2)),
                    ],
                    3: [
                        (2, qk_chunk(3, K, 3)),
                        (4, qk_chunk(1, Q, 2)),
                        (6, qk_chunk(1, Q, 3)),
                        (8, qk_chunk(2, Q, 2)),
                        (10, qk_chunk(2, Q, 3)),
                        (12, qk_chunk(3, Q, 2)),
                        (14, qk_chunk(3, Q, 3)),
                    ],
                    4: [(3, st3_chunk(0)), (7, st3_chunk(1)), (11, st3_chunk(2))],
                    5: [(3, st3_chunk(3)), (7, st3_chunk(4)), (11, st3_chunk(5))],
                    6: [(3, st3_chunk(6)), (9, st3_chunk(7))],
                    7: [],
                }

                eps_t = statp.tile([P, 1], F32, tag="eps", bufs=1)
                nc.vector.memset(eps_t, LN_EPS)

                for sh in range(2):
                    for pp in range(NP):
                        s0 = sh * 1024
                        blk = sh * NP + pp
                        slots = {}
                        for t, fn in sched.get(blk, []):
                            slots.setdefault(t, []).append(fn)
                        if blk == 0:
                            # V tile 0 before the loop: t=0's ctx needs it
                            v_chunk(0)()
                        cx = None
                        for t in range(ST):
                            for fn in slots.get(t, []):
                                fn()
                            sc = [
                                scp.tile([P, 1024], F32, tag="SC", name=f"sc{_hl}")
                                for _hl in range(2)
                            ]
                            for hl in range(2):
                                lo, hi = D * hl, D * (hl + 1)
                                for cc in range(2):
                                    nc.tensor.matmul(
                                        sc[hl][:, cc * 512 : (cc + 1) * 512],
                                        lhsT=KT[lo:hi, pp, t * P : (t + 1) * P],
                                        rhs=QT[
                                            lo:hi,
                                            pp,
                                            s0 + cc * 512 : s0 + (cc + 1) * 512,
                                        ],
                                        start=True,
                                        stop=True,
                                    )
                            et_t = expp.tile([P, 2048], BF16, tag="expT", name="et_t")
                            for hl in range(2):
                                nc.scalar.activation(
                                    out=et_t[:, hl * 1024 : (hl + 1) * 1024],
                                    in_=sc[hl],
                                    func=AF.Exp,
                                    scale=SCALE,
                                )
                            if cx is None:
                                # allocated after t=0's scores/exp so this
                                # block's first scores don't wait on the
                                # previous block's normalize chain
                                cx = [
                                    ctxp.tile(
                                        [D + 1, 1024], F32, tag="ctx", name=f"cx{_hl}"
                                    )
                                    for _hl in range(2)
                                ]
                            for hl in range(2):
                                h = 2 * pp + hl
                                for cc in range(2):
                                    nc.tensor.matmul(
                                        cx[hl][:, cc * 512 : (cc + 1) * 512],
                                        lhsT=Vaug[:, t, h, :],
                                        rhs=et_t[
                                            :,
                                            hl * 1024
                                            + cc * 512 : hl * 1024
                                            + (cc + 1) * 512,
                                        ],
                                        start=(t == 0),
                                        stop=(t == ST - 1),
                                    )
                        # softmax normalization: row D of cx is the denominator.
                        # reciprocal -> DRAM bounce -> zero-stride broadcast back
                        for hl in range(2):
                            rec = smallp.tile([P, 1024], F32, tag="rec", name="rec")
                            nc.vector.reciprocal(
                                rec[D : D + 1, :], cx[hl][D : D + 1, :]
                            )
                            dden = dramp.tile([1, 1024], F32, tag="dden", name="dden")
                            nc.sync.dma_start(out=dden, in_=rec[D : D + 1, :])
                            dbc = smallp.tile([D, 1024], F32, tag="dbc", name="dbc")
                            nc.gpsimd.dma_start(out=dbc, in_=_bcast_ap(dden[0], D))
                            if hl == 0:
                                nc.vector.tensor_tensor(
                                    CCT[0:D, pp, s0 : s0 + 1024],
                                    cx[hl][0:D, :],
                                    dbc,
                                    OP.mult,
                                )
                            else:
                                # result must land on partitions 64..127; DVE
                                # cannot shift partitions, DMA can.
                                tmp = smallp.tile(
                                    [D, 1024], BF16, tag="tmp", name="tmp"
                                )
                                nc.vector.tensor_tensor(
                                    tmp, cx[hl][0:D, :], dbc, OP.mult
                                )
                                nc.sync.dma_start(
                                    out=CCT[D : 2 * D, pp, s0 : s0 + 1024], in_=tmp
                                )
                        if blk == 1:
                            # stage-3 constants: emitted here so their DMAs
                            # never contend with the startup's critical loads
                            nc.gpsimd.dma_start(
                                out=Wo_sb,
                                in_=wopD[:].rearrange("(kt p) e -> p kt e", p=P),
                            )
                            for dram, sb in ((gammaD, gamma_bc), (betaD, beta_bc)):
                                nc.gpsimd.dma_start(out=sb, in_=_bcast_ap(dram[:], P))
                        if blk == 2:
                            # X fp32: only the output stage's residual reads it
                            xDr = xD[:].rearrange("(st p) e -> p st e", p=P)
                            for q in range(4):
                                nc.gpsimd.dma_start(
                                    out=X[:, 4 * q : 4 * q + 4],
                                    in_=xDr[:, 4 * q : 4 * q + 4],
                                )

            # ---------------- stage 3: Wo, residual, LayerNorm ----------------
            with (
                tc.tile_pool(name="outp3", bufs=6) as outp3,
                tc.tile_pool(name="ps3", bufs=6, space="PSUM") as ps3,
                tc.tile_pool(name="statp3", bufs=8) as statp3,
            ):
                eps_t = statp3.tile([P, 1], F32, tag="eps", bufs=1, name="eps_t3")
                nc.vector.memset(eps_t, LN_EPS)
                # deprioritized: fills engine-idle slots during the last
                # attention block instead of starving its scores
                tc.cur_priority += 20000
                for st in range(8, ST):
                    po = ps3.tile([P, E], F32, tag="po", name="po")
                    for kt in range(ET):
                        nc.tensor.matmul(
                            po,
                            lhsT=CCT[:, kt, st * P : (st + 1) * P],
                            rhs=Wo_sb[:, kt],
                            start=(kt == 0),
                            stop=False,
                        )
                    nc.tensor.matmul(
                        po, lhsT=ones_bf, rhs=bo_row, start=False, stop=True
                    )
                    y = outp3.tile([P, E], F32, tag="y", name="y")
                    nc.vector.tensor_add(y, po, X[:, st])
                    stats = statp3.tile([P, 6], F32, tag="stats", name="stats")
                    nc.vector.bn_stats(out=stats, in_=y)
                    mv = statp3.tile([P, 2], F32, tag="mv", name="mv")
                    nc.vector.bn_aggr(out=mv, in_=stats)
                    rstd = statp3.tile([P, 1], F32, tag="rstd", name="rstd")
                    # rstd = exp(-0.5*ln(var+eps)): Ln and Exp share one ACT
                    # table set, so no table reloads between softmax exps
                    nc.scalar.activation(
                        out=rstd, in_=mv[:, 1:2], func=AF.Ln, bias=eps_t
                    )
                    nc.scalar.activation(out=rstd, in_=rstd, func=AF.Exp, scale=-0.5)
                    nc.vector.tensor_scalar(
                        y, y, mv[:, 0:1], rstd, OP.subtract, OP.mult
                    )
                    nc.vector.tensor_tensor(y, y, gamma_bc, OP.mult)
                    nc.gpsimd.tensor_tensor(y, y, beta_bc, OP.add)
                    nc.sync.dma_start(out=outD[st * P : (st + 1) * P, :], in_=y)
                tc.cur_priority -= 20000

    _patch_to_json(nc)
    return nc


_NC_CACHE = None


def _get_nc():
    global _NC_CACHE
    if _NC_CACHE is None:
        _NC_CACHE = build_nc()
    return _NC_CACHE


def kernel(**inputs) -> np.ndarray:
    import ml_dtypes
    from concourse.bass_utils import run_bass_kernel_spmd

    BF = ml_dtypes.bfloat16
    nc = _get_nc()
    x = np.asarray(inputs["x"], dtype=np.float32)
    B = x.shape[0]

    def f32(k):
        return np.ascontiguousarray(np.asarray(inputs[k], dtype=np.float32))

    def perm_w(k):  # [H, E, D] -> [E, H*D] bf16
        w = np.asarray(inputs[k], dtype=np.float32)
        return np.ascontiguousarray(w.transpose(1, 0, 2).reshape(E, H * D).astype(BF))

    bqk = np.ascontiguousarray(
        np.stack(
            [
                np.asarray(inputs["bq"], np.float32).reshape(NP, P).T,
                np.asarray(inputs["bk"], np.float32).reshape(NP, P).T,
            ],
            axis=1,
        )
    )
    shared = {
        "Wq_p": perm_w("Wq"),
        "Wk_p": perm_w("Wk"),
        "Wv_p": perm_w("Wv"),
        "Wo_p": np.ascontiguousarray(
            np.asarray(inputs["Wo"], np.float32).astype(BF)
        ),
        "bqk": bqk,
        "bv": f32("bv"),
        "bo": f32("bo"),
        "gamma": f32("gamma"),
        "beta": f32("beta"),
    }
    in_maps = []
    for b in range(B):
        xb = np.ascontiguousarray(x[b])
        in_maps.append(
            {
                "x": xb,
                "xT": np.ascontiguousarray(xb.T.astype(BF)),
                **shared,
            }
        )
    res = run_bass_kernel_spmd(nc, in_maps, core_ids=list(range(B)))
    return np.stack([res.results[b]["out"] for b in range(B)], axis=0)


# revision 43
# speedup vs baseline: 1.0278x; 1.0093x over previous
"""MultiHeadAttention (8 heads, d_emb=512, d_hid=64, seq 2048, batch 8) on 8
Trainium2 NeuronCores.

Sharding: data parallel over batch — core i computes batch element i fully
(weights replicated, no collectives).

Per-core pipeline (engines overlap; ScalarE's 33.5M softmax exps are the
roofline):
  setup:   X loaded fp32 (kept for the residual), cast bf16, bounced through
           DRAM for DMA-transpose into X^T; weights cast bf16.
  Q/K:     per head-pair, heads col-stacked in the stationary operand so the
           projection matmuls run the full 128-wide array; per-partition bias
           add fused into the PSUM->SBUF eviction. Pair 0 up front; pairs 1-3
           stream through the scores PSUM slots inside the attention loop.
  V:       all heads at once (N=512), bias via a rank-1 (K=1 ones) matmul,
           stored with an appended ones column (V_aug) so the attention matmul
           also produces softmax denominators; interleaved into pair 0's loop.
  attn:    per (pair, s-half, key-tile): scores^T = K^T.T @ Q^T with both
           heads row-packed; exp(scale*x) on ScalarE straight out of PSUM into
           bf16 SBUF; ctx_aug^T += V_aug.T @ exp^T accumulated in PSUM.
           Normalization: row 64 of ctx_aug^T is the denominator; reciprocal,
           DRAM-bounce partition-broadcast, one multiply into concat^T.
  out:     out = concat^T.T @ Wo (+bo rank-1), residual add, LayerNorm via
           bn_stats/bn_aggr, DMA out.
"""

import copy
import json
import sys
import types

import numpy as np

for _p in ("/opt/trn_rl_repo", "/root/.axon_site/_ro/trn_rl_repo"):
    if _p not in sys.path:
        sys.path.append(_p)

import concourse.bass as bass
import concourse.mybir as mybir
import concourse.tile as tile

P = 128
S = 2048  # sequence length
E = 512  # embedding dim
H = 8  # heads
D = 64  # head dim
NP = H // 2  # head pairs
ST = S // P  # seq tiles
ET = E // P  # embedding tiles
SCALE = 1.0 / 8.0  # 1/sqrt(D)
LN_EPS = 1e-5
F32 = mybir.dt.float32
BF16 = mybir.dt.bfloat16
AF = mybir.ActivationFunctionType
OP = mybir.AluOpType


# --------------------------------------------------------------------------
# walrus in this build accepts only ONE sync-wait per instruction; Tile's sem
# assignment can attach several (e.g. the kernel-tail drain). Splitting the
# extra waits onto preceding NoOps on the same engine is semantically
# identical (engine streams execute in order).
def _split_waits(m, max_waits=1):
    for fn in m.get("functions", []):
        for blk in fn.get("blocks", []):
            new_insts = []
            for inst in blk.get("instructions", []):
                sync = inst.get("sync_info") or {}
                ow = sync.get("on_wait") or []
                if len(ow) > max_waits:
                    extra = ow[:-max_waits]
                    inst["sync_info"]["on_wait"] = ow[-max_waits:]
                    for ci in range(0, len(extra), max_waits):
                        nop = copy.deepcopy(inst)
                        nop["name"] = f"{inst['name']}ws{ci}"
                        nop["opcode"] = "NoOp"
                        nop["ins"] = []
                        nop["outs"] = []
                        nop["is_reset_sema"] = False
                        nop["sync_info"] = {
                            "on_update": [],
                            "on_wait": extra[ci : ci + max_waits],
                        }
                        new_insts.append(nop)
                new_insts.append(inst)
            blk["instructions"] = new_insts
    return m


def _patch_to_json(nc):
    orig = nc.to_json_bytes

    def patched(self):
        return json.dumps(_split_waits(json.loads(orig()))).encode()

    nc.to_json_bytes = types.MethodType(patched, nc)


def _bcast_ap(ap, parts):
    """[N]-shaped DRAM AP -> [parts, N] via zero-stride partition dim."""
    return bass.AP(
        tensor=ap.tensor, offset=ap.offset, ap=[[0, parts]] + list(ap.ap[-1:])
    )


def _emit_qk(nc, pool, pp, Wq_sb, Wk_sb, XT, QT, KT, bqk, psum_tag="pq"):
    for qk, wsb, qt in ((0, Wq_sb, QT), (1, Wk_sb, KT)):
        for cc in range(4):
            pq = pool.tile([P, 512], F32, tag=psum_tag, name="pq")
            for et in range(ET):
                nc.tensor.matmul(
                    pq,
                    lhsT=wsb[:, et, 2 * pp : 2 * pp + 2, :],
                    rhs=XT[:, et, cc * 512 : (cc + 1) * 512],
                    start=(et == 0),
                    stop=(et == ET - 1),
                )
            nc.vector.tensor_scalar_add(
                qt[:, pp, cc * 512 : (cc + 1) * 512], pq, bqk[:, qk, pp : pp + 1]
            )


# --------------------------------------------------------------------------
def build_nc():
    nc = bass.Bass()
    xD = nc.declare_dram_parameter("x", [S, E], F32, isOutput=False)
    bvD = nc.declare_dram_parameter("bv", [H, D], F32, isOutput=False)
    boD = nc.declare_dram_parameter("bo", [E], F32, isOutput=False)
    gammaD = nc.declare_dram_parameter("gamma", [E], F32, isOutput=False)
    betaD = nc.declare_dram_parameter("beta", [E], F32, isOutput=False)
    # host-preprocessed layouts: x^T and e-major weights, already bf16
    xTD = nc.declare_dram_parameter("xT", [E, S], BF16, isOutput=False)
    wqpD = nc.declare_dram_parameter("Wq_p", [E, H * D], BF16, isOutput=False)
    wkpD = nc.declare_dram_parameter("Wk_p", [E, H * D], BF16, isOutput=False)
    wvpD = nc.declare_dram_parameter("Wv_p", [E, H * D], BF16, isOutput=False)
    wopD = nc.declare_dram_parameter("Wo_p", [H * D, E], BF16, isOutput=False)
    bqkD = nc.declare_dram_parameter("bqk", [P, 2, NP], F32, isOutput=False)
    outD = nc.declare_dram_parameter("out", [S, E], F32, isOutput=True)

    with tile.TileContext(nc) as tc:
        with (
            tc.tile_pool(name="persist", bufs=1) as persist,
            tc.tile_pool(name="dramp", bufs=2, space="DRAM") as dramp,
        ):
            X = persist.tile([P, ST, E], F32, name="Xsb")
            XT = persist.tile([P, ET, S], BF16, name="XTsb")
            Wq_sb = persist.tile([P, ET, H, D], BF16, name="Wq_sb")
            Wk_sb = persist.tile([P, ET, H, D], BF16, name="Wk_sb")
            Wv_sb = persist.tile([P, ET, H, D], BF16, name="Wv_sb")
            Wo_sb = persist.tile([P, ET, E], BF16, name="Wo_sb")
            bqk = persist.tile([P, 2, NP], F32, name="bqk")
            bv_bc = persist.tile([P, H, D], F32, name="bv_bc")
            bo_row = persist.tile([1, E], BF16, name="bo_row")
            bo_stg = persist.tile([1, E], F32, name="bo_stg")
            ones_bf = persist.tile([1, P], BF16, name="ones_bf")
            gamma_bc = persist.tile([P, E], F32, name="gamma_bc")
            beta_bc = persist.tile([P, E], F32, name="beta_bc")
            QT = persist.tile([P, NP, S], BF16, name="QTsb")
            KT = persist.tile([P, NP, S], BF16, name="KTsb")
            Vaug = persist.tile([P, ST, H, D + 1], BF16, name="Vaug")
            CCT = persist.tile([P, NP, S], BF16, name="CCTsb")

            # ---------------- stage 0: direct loads (host pre-layouts) -------
            with (
                tc.tile_pool(name="qkp", bufs=2, space="PSUM") as qkp,
            ):
                nc.vector.memset(Vaug[:, :, :, D : D + 1], 1.0)
                nc.vector.memset(ones_bf, 1.0)

                # PE warmup during the initial DMA wait: HAM un-throttles
                # after ~3.4us of sustained activity, so the first real
                # matmuls (pair-0 Q/K) run at full clock instead of 1/2
                warm = qkp.tile([P, 512], F32, tag="pq", name="warm")
                for _w in range(350):
                    nc.tensor.matmul(
                        warm[:, 0:64], lhsT=ones_bf, rhs=ones_bf[:, 0:64],
                        start=True, stop=True,
                    )

                # critical chain first: x^T, Wq/Wk, biases -> pair-0 Q/K
                for et in range(ET):
                    nc.sync.dma_start(
                        out=XT[:, et], in_=xTD[et * P : (et + 1) * P, :]
                    )
                for wD, wsb in ((wqpD, Wq_sb), (wkpD, Wk_sb)):
                    nc.sync.dma_start(
                        out=wsb,
                        in_=wD[:].rearrange("(et p) hd -> p et hd", p=P).rearrange(
                            "p et (h d) -> p et h d", h=H
                        ),
                    )
                nc.sync.dma_start(out=bqk, in_=bqkD[:])
                _emit_qk(nc, qkp, 0, Wq_sb, Wk_sb, XT, QT, KT, bqk)

                # the rest, off the critical queue
                nc.sync.dma_start(
                    out=Wv_sb,
                    in_=wvpD[:].rearrange("(et p) hd -> p et hd", p=P).rearrange(
                        "p et (h d) -> p et h d", h=H
                    ),
                )

                nc.gpsimd.dma_start(
                    out=bv_bc.rearrange("p h d -> p (h d)"),
                    in_=_bcast_ap(bvD[:].rearrange("h d -> (h d)"), P),
                )
                nc.gpsimd.dma_start(out=bo_stg, in_=boD[:][None, :])
                nc.gpsimd.tensor_copy(out=bo_row, in_=bo_stg)

            # ---------------- stage 2: attention ----------------
            with (
                tc.tile_pool(name="expp", bufs=6) as expp,
                tc.tile_pool(name="scp", bufs=2, space="PSUM") as scp,
                tc.tile_pool(name="ctxp", bufs=2, space="PSUM") as ctxp,
                tc.tile_pool(name="smallp", bufs=3) as smallp,
                tc.tile_pool(name="outp", bufs=3) as outp,
                tc.tile_pool(name="statp", bufs=4) as statp,
            ):
                # deferred work, interleaved through the scores PSUM slots:
                # V tiles during (pair0, sh0); Q/K of pair p+1 during later blocks
                def v_chunk(st):
                    def emit():
                        pv = scp.tile([P, 1024], F32, tag="SC", name="pv")
                        for et in range(ET):
                            nc.tensor.matmul(
                                pv[:, 0:512],
                                lhsT=XT[:, et, st * P : (st + 1) * P],
                                rhs=Wv_sb[:, et],
                                start=(et == 0),
                                stop=(et == ET - 1),
                            )
                        nc.vector.tensor_tensor(
                            Vaug[:, st, :, 0:D],
                            pv[:, 0:512].rearrange("p (h d) -> p h d", h=H),
                            bv_bc,
                            OP.add,
                        )

                    return emit

                def qk_chunk(pp, qk, cc):
                    def emit():
                        wsb = Wq_sb if qk == 0 else Wk_sb
                        qt = QT if qk == 0 else KT
                        pq = scp.tile([P, 1024], F32, tag="SC", name="pq2")
                        for et in range(ET):
                            nc.tensor.matmul(
                                pq[:, 0:512],
                                lhsT=wsb[:, et, 2 * pp : 2 * pp + 2, :],
                                rhs=XT[:, et, cc * 512 : (cc + 1) * 512],
                                start=(et == 0),
                                stop=(et == ET - 1),
                            )
                        nc.vector.tensor_scalar_add(
                            qt[:, pp, cc * 512 : (cc + 1) * 512],
                            pq[:, 0:512],
                            bqk[:, qk, pp : pp + 1],
                        )

                    return emit

                # explicit chunk schedule (sh-outer block order):
                # every chunk lands before its first consumer, spread so PE
                # stays under ScalarE's per-iteration budget
                Q, K = 0, 1

                def st3_chunk(st):
                    def emit():
                        po = scp.tile([P, 1024], F32, tag="SC", name="po3")
                        for kt in range(ET):
                            nc.tensor.matmul(
                                po[:, 0:E],
                                lhsT=CCT[:, kt, st * P : (st + 1) * P],
                                rhs=Wo_sb[:, kt],
                                start=(kt == 0),
                                stop=False,
                            )
                        nc.tensor.matmul(
                            po[:, 0:E], lhsT=ones_bf, rhs=bo_row, start=False, stop=True
                        )
                        y = outp.tile([P, E], F32, tag="y", name="y")
                        nc.vector.tensor_add(y, po[:, 0:E], X[:, st])
                        stats = statp.tile([P, 6], F32, tag="stats", name="stats")
                        nc.vector.bn_stats(out=stats, in_=y)
                        mv = statp.tile([P, 2], F32, tag="mv", name="mv")
                        nc.vector.bn_aggr(out=mv, in_=stats)
                        rstd = statp.tile([P, 1], F32, tag="rstd", name="rstd")
                        nc.scalar.activation(
                            out=rstd, in_=mv[:, 1:2], func=AF.Ln, bias=eps_t
                        )
                        nc.scalar.activation(
                            out=rstd, in_=rstd, func=AF.Exp, scale=-0.5
                        )
                        nc.vector.tensor_scalar(
                            y, y, mv[:, 0:1], rstd, OP.subtract, OP.mult
                        )
                        nc.vector.tensor_tensor(y, y, gamma_bc, OP.mult)
                        nc.vector.tensor_tensor(y, y, beta_bc, OP.add)
                        nc.sync.dma_start(out=outD[st * P : (st + 1) * P, :], in_=y)

                    return emit

                sched = {
                    0: [(t, v_chunk(t)) for t in range(1, 16)]
                    + [
                        (13, qk_chunk(1, Q, 0)),
                        (14, qk_chunk(1, Q, 1)),
                        (15, qk_chunk(1, K, 0)),
                    ],
                    1: [
                        (2, qk_chunk(1, K, 1)),
                        (4, qk_chunk(1, K, 2)),
                        (6, qk_chunk(1, K, 3)),
                        (8, qk_chunk(2, Q, 0)),
                        (10, qk_chunk(2, Q, 1)),
                        (12, qk_chunk(2, K, 0)),
                        (14, qk_chunk(2, K, 1)),
                    ],
                    2: [
                        (2, qk_chunk(2, K, 2)),
                        (4, qk_chunk(2, K, 3)),
                        (6, qk_chunk(3, Q, 0)),
                        (8, qk_chunk(3, Q, 1)),
                        (10, qk_chunk(3, K, 0)),
                        (12, qk_chunk(3, K, 1)),
                        (14, qk_chunk(3, K, 2)),
                    ],
                    3: [
                        (2, qk_chunk(3, K, 3)),
                        (4, qk_chunk(1, Q, 2)),
                        (6, qk_chunk(1, Q, 3)),
                        (8, qk_chunk(2, Q, 2)),
                        (10, qk_chunk(2, Q, 3)),
                        (12, qk_chunk(3, Q, 2)),
                        (14, qk_chunk(3, Q, 3)),
                    ],
                    4: [(3, st3_chunk(0)), (7, st3_chunk(1)), (11, st3_chunk(2))],
                    5: [(3, st3_chunk(3)), (7, st3_chunk(4)), (11, st3_chunk(5))],
                    6: [(3, st3_chunk(6)), (9, st3_chunk(7))],
                    7: [],
                }

                eps_t = statp.tile([P, 1], F32, tag="eps", bufs=1)
                nc.vector.memset(eps_t, LN_EPS)

                for sh in range(2):
                    for pp in range(NP):
                        s0 = sh * 1024
                        blk = sh * NP + pp
                        slots = {}
                        for t, fn in sched.get(blk, []):
                            slots.setdefault(t, []).append(fn)
                        if blk == 0:
                            # V tile 0 before the loop: t=0's ctx needs it
                            v_chunk(0)()
                        cx = None
                        for t in range(ST):
                            for fn in slots.get(t, []):
                                fn()
                            sc = [
                                scp.tile([P, 1024], F32, tag="SC", name=f"sc{_hl}")
                                for _hl in range(2)
                            ]
                            for hl in range(2):
                                lo, hi = D * hl, D * (hl + 1)
                                for cc in range(2):
                                    nc.tensor.matmul(
                                        sc[hl][:, cc * 512 : (cc + 1) * 512],
                                        lhsT=KT[lo:hi, pp, t * P : (t + 1) * P],
                                        rhs=QT[
                                            lo:hi,
                                            pp,
                                            s0 + cc * 512 : s0 + (cc + 1) * 512,
                                        ],
                                        start=True,
                                        stop=True,
                                    )
                            et_t = expp.tile([P, 2048], BF16, tag="expT", name="et_t")
                            for hl in range(2):
                                nc.scalar.activation(
                                    out=et_t[:, hl * 1024 : (hl + 1) * 1024],
                                    in_=sc[hl],
                                    func=AF.Exp,
                                    scale=SCALE,
                                )
                            if cx is None:
                                # allocated after t=0's scores/exp so this
                                # block's first scores don't wait on the
                                # previous block's normalize chain
                                cx = [
                                    ctxp.tile(
                                        [D + 1, 1024], F32, tag="ctx", name=f"cx{_hl}"
                                    )
                                    for _hl in range(2)
                                ]
                            for hl in range(2):
                                h = 2 * pp + hl
                                for cc in range(2):
                                    nc.tensor.matmul(
                                        cx[hl][:, cc * 512 : (cc + 1) * 512],
                                        lhsT=Vaug[:, t, h, :],
                                        rhs=et_t[
                                            :,
                                            hl * 1024
                                            + cc * 512 : hl * 1024
                                            + (cc + 1) * 512,
                                        ],
                                        start=(t == 0),
                                        stop=(t == ST - 1),
                                    )
                        # softmax normalization: row D of cx is the denominator.
                        # reciprocal -> DRAM bounce -> zero-stride broadcast back
                        for hl in range(2):
                            rec = smallp.tile([P, 1024], F32, tag="rec", name="rec")
                            nc.vector.reciprocal(
                                rec[D : D + 1, :], cx[hl][D : D + 1, :]
                            )
                            dden = dramp.tile([1, 1024], F32, tag="dden", name="dden")
                            nc.sync.dma_start(out=dden, in_=rec[D : D + 1, :])
                            dbc = smallp.tile([D, 1024], F32, tag="dbc", name="dbc")
                            nc.gpsimd.dma_start(out=dbc, in_=_bcast_ap(dden[0], D))
                            if hl == 0:
                                nc.vector.tensor_tensor(
                                    CCT[0:D, pp, s0 : s0 + 1024],
                                    cx[hl][0:D, :],
                                    dbc,
                                    OP.mult,
                                )
                            else:
                                # result must land on partitions 64..127; DVE
                                # cannot shift partitions, DMA can.
                                tmp = smallp.tile(
                                    [D, 1024], BF16, tag="tmp", name="tmp"
                                )
                                nc.vector.tensor_tensor(
                                    tmp, cx[hl][0:D, :], dbc, OP.mult
                                )
                                nc.sync.dma_start(
                                    out=CCT[D : 2 * D, pp, s0 : s0 + 1024], in_=tmp
                                )
                        if blk == 1:
                            # stage-3 constants: emitted here so their DMAs
                            # never contend with the startup's critical loads
                            nc.gpsimd.dma_start(
                                out=Wo_sb,
                                in_=wopD[:].rearrange("(kt p) e -> p kt e", p=P),
                            )
                            for dram, sb in ((gammaD, gamma_bc), (betaD, beta_bc)):
                                nc.gpsimd.dma_start(out=sb, in_=_bcast_ap(dram[:], P))
                        if blk == 2:
                            # X fp32: only the output stage's residual reads it
                            xDr = xD[:].rearrange("(st p) e -> p st e", p=P)
                            for q in range(4):
                                nc.gpsimd.dma_start(
                                    out=X[:, 4 * q : 4 * q + 4],
                                    in_=xDr[:, 4 * q : 4 * q + 4],
                                )

            # ---------------- stage 3: Wo, residual, LayerNorm ----------------
            with (
                tc.tile_pool(name="outp3", bufs=6) as outp3,
                tc.tile_pool(name="ps3", bufs=6, space="PSUM") as ps3,
                tc.tile_pool(name="statp3", bufs=8) as statp3,
            ):
                eps_t = statp3.tile([P, 1], F32, tag="eps", bufs=1, name="eps_t3")
                nc.vector.memset(eps_t, LN_EPS)
                # deprioritized: fills engine-idle slots during the last
                # attention block instead of starving its scores
                tc.cur_priority += 20000
                for st in range(8, ST):
                    po = ps3.tile([P, E], F32, tag="po", name="po")
                    for kt in range(ET):
                        nc.tensor.matmul(
                            po,
                            lhsT=CCT[:, kt, st * P : (st + 1) * P],
                            rhs=Wo_sb[:, kt],
                            start=(kt == 0),
                            stop=False,
                        )
                    nc.tensor.matmul(
                        po, lhsT=ones_bf, rhs=bo_row, start=False, stop=True
                    )
                    y = outp3.tile([P, E], F32, tag="y", name="y")
                    nc.vector.tensor_add(y, po, X[:, st])
                    stats = statp3.tile([P, 6], F32, tag="stats", name="stats")
                    nc.vector.bn_stats(out=stats, in_=y)
                    mv = statp3.tile([P, 2], F32, tag="mv", name="mv")
                    nc.vector.bn_aggr(out=mv, in_=stats)
                    rstd = statp3.tile([P, 1], F32, tag="rstd", name="rstd")
                    # rstd = exp(-0.5*ln(var+eps)): Ln and Exp share one ACT
                    # table set, so no table reloads between softmax exps
                    nc.scalar.activation(
                        out=rstd, in_=mv[:, 1:2], func=AF.Ln, bias=eps_t
                    )
                    nc.scalar.activation(out=rstd, in_=rstd, func=AF.Exp, scale=-0.5)
                    nc.vector.tensor_scalar(
                        y, y, mv[:, 0:1], rstd, OP.subtract, OP.mult
                    )
                    nc.vector.tensor_tensor(y, y, gamma_bc, OP.mult)
                    nc.gpsimd.tensor_tensor(y, y, beta_bc, OP.add)
                    nc.sync.dma_start(out=outD[st * P : (st + 1) * P, :], in_=y)
                tc.cur_priority -= 20000

    _patch_to_json(nc)
    return nc


_NC_CACHE = None


def _get_nc():
    global _NC_CACHE
    if _NC_CACHE is None:
        _NC_CACHE = build_nc()
    return _NC_CACHE


def kernel(**inputs) -> np.ndarray:
    import ml_dtypes
    from concourse.bass_utils import run_bass_kernel_spmd

    BF = ml_dtypes.bfloat16
    nc = _get_nc()
    x = np.asarray(inputs["x"], dtype=np.float32)
    B = x.shape[0]

    def f32(k):
        return np.ascontiguousarray(np.asarray(inputs[k], dtype=np.float32))

    def perm_w(k):  # [H, E, D] -> [E, H*D] bf16
        w = np.asarray(inputs[k], dtype=np.float32)
        return np.ascontiguousarray(w.transpose(1, 0, 2).reshape(E, H * D).astype(BF))

    bqk = np.ascontiguousarray(
        np.stack(
            [
                np.asarray(inputs["bq"], np.float32).reshape(NP, P).T,
                np.asarray(inputs["bk"], np.float32).reshape(NP, P).T,
            ],
            axis=1,
        )
    )
    shared = {
        "Wq_p": perm_w("Wq"),
        "Wk_p": perm_w("Wk"),
        "Wv_p": perm_w("Wv"),
        "Wo_p": np.ascontiguousarray(
            np.asarray(inputs["Wo"], np.float32).astype(BF)
        ),
        "bqk": bqk,
        "bv": f32("bv"),
        "bo": f32("bo"),
        "gamma": f32("gamma"),
        "beta": f32("beta"),
    }
    in_maps = []
    for b in range(B):
        xb = np.ascontiguousarray(x[b])
        in_maps.append(
            {
                "x": xb,
                "xT": np.ascontiguousarray(xb.T.astype(BF)),
                **shared,
            }
        )
    res = run_bass_kernel_spmd(nc, in_maps, core_ids=list(range(B)))
    return np.stack([res.results[b]["out"] for b in range(B)], axis=0)
